# revision 1
# baseline (speedup 1.0000x reference)
"""GNN message-passing (CompGCN edge-softmax) TRN2 kernel — no rel gather.

Same contract/sharding as kernel.py (edges sharded by dst range, 8 cores).

SWDGE gather costs ~8 ns/row on GPSIMD, so this version gathers ONLY
ent[src] (one row per edge).  The other two per-edge rows come from TensorE:

  dst rows:  dstrows[e,h] = sum_j OHT[j,e]*E_blk[j,h]   (E_blk = block's 128
             local dst rows, contiguous DMA, dual-bf16 hi+lo for exactness)
  rel rows:  relrows[e,h] = sum_r rOHT[r',e]*rel_q[r',h] (rel table SBUF-
             resident in 8 chunks of 128 rows, dual-bf16 hi+lo; edges are
             rel-sorted within each block section with per-(section,q) slot
             counts FIXED across cores, so each 128-slot chunk intersects a
             couple of compile-time-known q ranges -> partition-offset
             matmuls)

score = sum(comp * dstrows) stays fp32-exact (dual-bf16 residual ~1e-4).
Aggregation runs in bf16 (comp cast + W one-hot*es), err ~1e-2 < 2e-2 tol:
    accT[h,j] += comp_c.T @ W_c      (bf16 TensorE, fp32 PSUM)
    den[j]     = Wsum.T @ ones       (Wsum = sum_c W_c on DVE)
    out_block  = tanh((accT.T @ neigh_w)/den)
"""

import numpy as np

N_ENT = 50000
N_REL = 1000
H = 128
P = 128
NQ = 8                      # rel table chunks of 128 rows
N_CORES = 8
NPC = N_ENT // N_CORES
LO_ROWS = 32768

_cache = {}


def _bfsplit(x):
    import ml_dtypes
    hi = x.astype(np.float32).astype(ml_dtypes.bfloat16)
    lo = (x.astype(np.float32) - hi.astype(np.float32)).astype(ml_dtypes.bfloat16)
    return hi, lo


def _build_program(npc, n_ent, lo_rows, s_los, s_his, runs_all, n_par):
    """runs_all[b] = list of (c, q, s0, s1); q%n_par picks the parity
    one-hot tile whose rows are zero outside q's slots."""
    import concourse.bacc as bacc
    import concourse.mybir as mybir
    import concourse.tile as tile

    f32 = mybir.dt.float32
    f16 = mybir.dt.float16
    bf16 = mybir.dt.bfloat16
    i16 = mybir.dt.int16
    n_blocks = len(s_los)
    s_tot = [a + b for a, b in zip(s_los, s_his)]
    S_max = max(s_tot)
    W_chunks = sum(s_tot)
    W_slots = W_chunks * P

    nc = bacc.Bacc("TRN2", target_bir_lowering=False, debug=False,
                   num_devices=N_CORES)

    ent = nc.dram_tensor("ent", [n_ent, H], f32, kind="ExternalInput")
    elh_in = nc.dram_tensor("ent_loc_hi", [npc, H], bf16, kind="ExternalInput")
    ell_in = nc.dram_tensor("ent_loc_lo", [npc, H], bf16, kind="ExternalInput")
    rlh_in = nc.dram_tensor("rel_hi", [P, NQ, H], bf16, kind="ExternalInput")
    rll_in = nc.dram_tensor("rel_lo", [P, NQ, H], bf16, kind="ExternalInput")
    w_in = nc.dram_tensor("w", [H, H], f32, kind="ExternalInput")
    iota_in = nc.dram_tensor("iota", [P, P], f16, kind="ExternalInput")
    ioc16_in = nc.dram_tensor("iota_col16", [P, 1], f16, kind="ExternalInput")
    sgi_in = nc.dram_tensor("src_gi", [P, W_chunks * 8], i16,
                            kind="ExternalInput")
    doh_in = nc.dram_tensor("dst_oh", [P, W_chunks], f16,
                            kind="ExternalInput")
    dohT_in = nc.dram_tensor("dst_ohT", [P, W_slots], f16,
                             kind="ExternalInput")
    ridT_in = [nc.dram_tensor(f"relidT{p}", [P, W_slots], f16,
                              kind="ExternalInput") for p in range(n_par)]
    out = nc.dram_tensor("out", [npc, H], f32, kind="ExternalOutput")

    import concourse.bass as bass

    def bc(ap, dims):
        return bass.AP(ap.tensor, ap.offset, dims)

    with tile.TileContext(nc) as tc:
        with (
            tc.tile_pool(name="const", bufs=1) as constp,
            tc.tile_pool(name="idx", bufs=1) as idxp,
            tc.tile_pool(name="data", bufs=2) as datap,
            tc.tile_pool(name="small", bufs=2) as smallp,
            tc.tile_pool(name="psum", bufs=1, space="PSUM") as psump,
            tc.tile_pool(name="psumr", bufs=2, space="PSUM") as psumrp,
            tc.tile_pool(name="psumb", bufs=1, space="PSUM") as psumbp,
        ):
            iota_t = constp.tile([P, P], f16)
            nc.sync.dma_start(iota_t[:], iota_in[:])
            ioc16_t = constp.tile([P, 1], f16)
            nc.sync.dma_start(ioc16_t[:], ioc16_in[:])
            w_t = constp.tile([H, H], f32)
            nc.sync.dma_start(w_t[:], w_in[:])
            rlh_t = constp.tile([P, NQ, H], bf16)
            nc.sync.dma_start(rlh_t[:], rlh_in[:])
            rll_t = constp.tile([P, NQ, H], bf16)
            nc.sync.dma_start(rll_t[:], rll_in[:])
            ones_bf = constp.tile([P, 1], bf16)
            nc.vector.memset(ones_bf[:], 1.0)

            sgi_t = idxp.tile([P, W_chunks * 8], i16)
            nc.sync.dma_start(sgi_t[:], sgi_in[:])
            doh_t = idxp.tile([P, W_chunks], f16)
            nc.sync.dma_start(doh_t[:], doh_in[:])

            coff = 0
            for b in range(n_blocks):
                base = b * P
                nodes_b = min(P, npc - base)
                s_lo, s_hi = s_los[b], s_his[b]
                S = s_lo + s_hi
                ns = S * P

                src_rows = datap.tile([P, S_max, H], f32, tag="src")
                relrows = datap.tile([P, S_max, H], f32, tag="relrows")
                w_oh = datap.tile([P, S_max, H], bf16, tag="W")
                comp_bf = datap.tile([P, S_max, H], bf16, tag="compbf")
                w_bf = datap.tile([P, S_max, H], bf16, tag="Wbf")
                oht_t = datap.tile([P, S_max * P], bf16, tag="OHT")
                roht_t = [datap.tile([P, S_max * P], bf16, tag=f"rOHT{p}",
                                     name=f"roht{p}") for p in range(n_par)]
                dohT_t = datap.tile([P, S_max * P], f16, tag="dohT")
                ridT_t = [datap.tile([P, S_max * P], f16, tag=f"ridT{p}",
                                     name=f"ridt{p}") for p in range(n_par)]
                ehi_t = datap.tile([P, H], bf16, tag="ehi")
                elo_t = datap.tile([P, H], bf16, tag="elo")

                if s_lo > 0:
                    nc.gpsimd.dma_gather(
                        src_rows[:, 0:s_lo, :], ent[0:lo_rows, :],
                        sgi_t[:, coff * 8:(coff + s_lo) * 8],
                        s_lo * P, s_lo * P, H, single_packet=False)
                if s_hi > 0:
                    nc.gpsimd.dma_gather(
                        src_rows[:, s_lo:S, :], ent[lo_rows:n_ent, :],
                        sgi_t[:, (coff + s_lo) * 8:(coff + S) * 8],
                        s_hi * P, s_hi * P, H, single_packet=False)
                nc.sync.dma_start(dohT_t[:, 0:ns],
                                  dohT_in[:, coff * P:coff * P + ns])
                for p in range(n_par):
                    nc.sync.dma_start(ridT_t[p][:, 0:ns],
                                      ridT_in[p][:, coff * P:coff * P + ns])
                if nodes_b < P:
                    nc.vector.memset(ehi_t[:], 0.0)
                    nc.vector.memset(elo_t[:], 0.0)
                nc.sync.dma_start(ehi_t[:nodes_b, :],
                                  elh_in[base:base + nodes_b, :])
                nc.sync.dma_start(elo_t[:nodes_b, :],
                                  ell_in[base:base + nodes_b, :])

                # transposed one-hots (bf16 out)
                i16_ap = ioc16_t[:]
                nc.vector.tensor_tensor(
                    out=oht_t[:, 0:ns], in0=dohT_t[:, 0:ns],
                    in1=bc(i16_ap, [i16_ap.ap[0], [0, ns]]),
                    op=mybir.AluOpType.is_equal)
                for p in range(n_par):
                    nc.vector.tensor_tensor(
                        out=roht_t[p][:, 0:ns], in0=ridT_t[p][:, 0:ns],
                        in1=bc(i16_ap, [i16_ap.ap[0], [0, ns]]),
                        op=mybir.AluOpType.is_equal)

                # dstrows[e,h] = OHT_c.T @ (E_hi + E_lo)
                drows_ps = psumbp.tile([P, S_max, H], f32, tag="drows")
                for c in range(S):
                    lhs = oht_t[:, c * P:(c + 1) * P]
                    nc.tensor.matmul(drows_ps[:, c, :], lhsT=lhs,
                                     rhs=ehi_t[:], start=True, stop=False)
                    nc.tensor.matmul(drows_ps[:, c, :], lhsT=lhs,
                                     rhs=elo_t[:], start=False, stop=True)

                # relrows chunks: accumulate one (hi+lo) mm pair per q
                # present in the chunk, via its parity one-hot tile
                for c in range(S):
                    rel_ps = psumrp.tile([P, H], f32, tag="relps")
                    qs = [q for (cc, q, _s0, _s1) in runs_all[b] if cc == c]
                    for i, q in enumerate(qs):
                        lhs = roht_t[q % n_par][:, c * P:(c + 1) * P]
                        nc.tensor.matmul(rel_ps[:], lhsT=lhs,
                                         rhs=rlh_t[:, q, :],
                                         start=(i == 0), stop=False)
                        nc.tensor.matmul(rel_ps[:], lhsT=lhs,
                                         rhs=rll_t[:, q, :],
                                         start=False, stop=(i == len(qs) - 1))
                    nc.scalar.copy(relrows[:, c, :], rel_ps[:])

                # comp (fp32, in-place over src_rows) + bf16 cast for accT
                nc.vector.tensor_tensor(
                    out=src_rows[:, 0:S, :], in0=src_rows[:, 0:S, :],
                    in1=relrows[:, 0:S, :], op=mybir.AluOpType.mult)
                nc.scalar.copy(comp_bf[:, 0:S, :], src_rows[:, 0:S, :])

                # score = sum_h comp*dstrows  (prod scratch into relrows)
                nc.vector.tensor_tensor(
                    out=relrows[:, 0:S, :], in0=src_rows[:, 0:S, :],
                    in1=drows_ps[:, 0:S, :], op=mybir.AluOpType.mult)
                score = smallp.tile([P, S_max], f32, tag="score")
                nc.vector.tensor_reduce(
                    out=score[:, 0:S], in_=relrows[:, 0:S, :],
                    axis=mybir.AxisListType.X, op=mybir.AluOpType.add)
                es = smallp.tile([P, S_max], bf16, tag="es")
                nc.scalar.activation(
                    out=es[:, 0:S], in_=score[:, 0:S],
                    func=mybir.ActivationFunctionType.Exp)

                # W one-hot (fp32) * es -> bf16
                doh_ap = doh_t[:, coff:coff + S]
                doh_b = bc(doh_ap, [doh_ap.ap[0], doh_ap.ap[1], [0, H]])
                iota_ap = iota_t[:]
                iota_b = bc(iota_ap, [iota_ap.ap[0], [0, S], iota_ap.ap[1]])
                nc.vector.tensor_tensor(
                    out=w_oh[:, 0:S, :], in0=doh_b, in1=iota_b,
                    op=mybir.AluOpType.is_equal)
                es_ap = es[:, 0:S]
                es_b = bc(es_ap, [es_ap.ap[0], es_ap.ap[1], [0, H]])
                nc.vector.tensor_tensor(
                    out=w_bf[:, 0:S, :], in0=w_oh[:, 0:S, :], in1=es_b,
                    op=mybir.AluOpType.mult)

                # den[j] = sum_c W_c.T @ ones  (bf16 TensorE)
                ps_m = psump.tile([P, H], f32, tag="misc")
                for c in range(S):
                    nc.tensor.matmul(ps_m[:, 0:1], lhsT=w_bf[:, c, :],
                                     rhs=ones_bf[:],
                                     start=(c == 0), stop=(c == S - 1))

                # accT[h,j] += comp_c.T @ W_c  (bf16)
                acct_ps = psump.tile([P, P], f32, tag="accT")
                for c in range(S):
                    nc.tensor.matmul(
                        acct_ps[:], lhsT=comp_bf[:, c, :], rhs=w_bf[:, c, :],
                        start=(c == 0), stop=(c == S - 1))

                acct_sb = smallp.tile([P, P], f32, tag="acct_sb")
                nc.scalar.copy(acct_sb[:], acct_ps[:])
                den_sb = smallp.tile([P, 1], f32, tag="den_sb")
                nc.vector.tensor_scalar_max(den_sb[:], ps_m[:, 0:1], 1e-30)
                rden = smallp.tile([P, 1], f32, tag="rden")
                nc.vector.reciprocal(rden[:], den_sb[:])

                nc.tensor.matmul(ps_m[:], lhsT=acct_sb[:], rhs=w_t[:],
                                 start=True, stop=True)
                out_sb = smallp.tile([P, H], f32, tag="out_sb")
                nc.scalar.activation(
                    out=out_sb[:], in_=ps_m[:],
                    func=mybir.ActivationFunctionType.Tanh, scale=rden[:])
                nc.sync.dma_start(out[base:base + nodes_b, :],
                                  out_sb[:nodes_b, :])
                coff += S

    nc.compile()
    return nc


def _idx_to_gather_layout(arr):
    a = arr.reshape(-1, 16).T.astype(np.int16)
    return np.tile(a, (8, 1))


def _prep_inputs(ent_emb, rel_emb, neigh_w, src, dst, rel_id):
    """Edges by dst block; rel-q-partitioned sections with core-uniform
    slot layout; build src gather idx + one-hot index maps."""
    import ml_dtypes
    src = np.asarray(src).astype(np.int64)
    dst = np.asarray(dst).astype(np.int64)
    rel_id = np.asarray(rel_id).astype(np.int64)
    n_blocks = (NPC + P - 1) // P

    order = np.argsort(dst, kind="stable")
    src_s, dst_s, rel_s = src[order], dst[order], rel_id[order]
    g_s = (dst_s // NPC) * n_blocks + (dst_s % NPC) // P
    n_gblocks = N_CORES * n_blocks
    bounds = np.searchsorted(g_s, np.arange(n_gblocks + 1))

    # per (core, block, section, q) edge lists
    # section 0 = src<LO_ROWS, 1 = src>=LO_ROWS
    per = {}
    for c in range(N_CORES):
        for b in range(n_blocks):
            e0, e1 = bounds[c * n_blocks + b], bounds[c * n_blocks + b + 1]
            s_g, d_g, r_g = src_s[e0:e1], dst_s[e0:e1], rel_s[e0:e1]
            sec = (s_g >= LO_ROWS).astype(np.int64)
            q_g = r_g // P
            for s in (0, 1):
                for q in range(NQ):
                    m = (sec == s) & (q_g == q)
                    per[(c, b, s, q)] = (s_g[m], d_g[m], r_g[m])

    # core-uniform slot counts per (block, section, q), 16-aligned for tidiness
    cnt = {}
    for b in range(n_blocks):
        for s in (0, 1):
            for q in range(NQ):
                m = max(len(per[(c, b, s, q)][0]) for c in range(N_CORES))
                if s == 0 and q == 0:
                    m = max(m, 1)
                cnt[(b, s, q)] = m

    # slot layout per block: lo qs, pad to 128; hi qs, pad to 128
    s_los, s_his, runs_all, layouts = [], [], [], []
    for b in range(n_blocks):
        lo_n = sum(cnt[(b, 0, q)] for q in range(NQ))
        hi_n = sum(cnt[(b, 1, q)] for q in range(NQ))
        s_lo = max((lo_n + P - 1) // P, 1)
        s_hi = (hi_n + P - 1) // P
        s_los.append(s_lo)
        s_his.append(s_hi)
        # slot ranges: [(q, s0, s1, sec)] in slot order; section pads get
        # q = last q of the section (repeat edges carry that q's relid)
        lay = []
        pos = 0
        for s, sbase, stot in ((0, 0, s_lo), (1, s_lo, s_hi)):
            pos = sbase * P
            for q in range(NQ):
                n = cnt[(b, s, q)]
                if n:
                    lay.append((q, pos, pos + n, s, False))
                    pos += n
            end = (sbase + stot) * P
            if pos < end and lay:
                lq, ls0, _ls1, lsec, lpad = lay[-1]
                if lsec == s and not lpad:
                    # extend the last real range; fill pads it with repeats
                    lay[-1] = (lq, ls0, end, s, False)
                else:
                    lay.append((lq, pos, end, s, True))
        layouts.append(lay)
        # runs: intersect layout ranges with 128-chunks
        runs = []
        for (q, s0, s1, _sec, _pad) in lay:
            c0, c1 = s0 // P, (s1 - 1) // P
            for c in range(c0, c1 + 1):
                a = max(s0, c * P)
                z = min(s1, (c + 1) * P)
                if a < z:
                    runs.append((c, q, a, z))
        runs_all.append(runs)
    s_tot = [a + b for a, b in zip(s_los, s_his)]
    W_chunks = sum(s_tot)

    iota = np.broadcast_to(np.arange(P, dtype=np.float16), (P, P)).copy()
    iota_col16 = np.arange(P, dtype=np.float16).reshape(P, 1).copy()

    # smallest parity count with no per-chunk q%k collision
    n_par = 3
    while True:
        ok = True
        for runs in runs_all:
            from collections import defaultdict
            byc = defaultdict(list)
            for (c, q, _s0, _s1) in runs:
                byc[c].append(q % n_par)
            if any(len(v) != len(set(v)) for v in byc.values()):
                ok = False
                break
        if ok:
            break
        n_par += 1
    rel_pad = np.zeros((NQ * P, H), np.float32)
    rel_pad[:N_REL] = np.asarray(rel_emb, np.float32)
    rhi, rlo = _bfsplit(rel_pad)
    rlh = np.ascontiguousarray(rhi.reshape(NQ, P, H).transpose(1, 0, 2))
    rll = np.ascontiguousarray(rlo.reshape(NQ, P, H).transpose(1, 0, 2))

    in_maps = []
    for cidx in range(N_CORES):
        sgi = np.zeros((W_chunks * P,), np.int16)
        doh = np.full((W_chunks * P,), float(P), np.float16)
        rid = np.zeros((W_chunks * P,), np.float16)
        qof = np.full((W_chunks * P,), -1, np.int64)
        coff = 0
        for b in range(n_blocks):
            o0 = coff * P
            for (q, s0, s1, sec, is_pad) in layouts[b]:
                ss, dd, rr = per[(cidx, b, sec, q)]
                if is_pad:
                    ss = ss[:0]
                    dd = dd[:0]
                    rr = rr[:0]
                n = len(ss)
                cap = s1 - s0
                assert n <= cap
                sub = LO_ROWS if sec == 1 else 0
                base = cidx * NPC + b * P
                qof[o0 + s0:o0 + s1] = q
                if n:
                    sgi[o0 + s0:o0 + s0 + n] = ss - sub
                    doh[o0 + s0:o0 + s0 + n] = (dd - base).astype(np.float16)
                    rid[o0 + s0:o0 + s0 + n] = (rr - q * P).astype(np.float16)
                if n < cap:
                    # pad: repeat a real edge of this q (doh stays 128),
                    # or r'=0 of this q if the core has none
                    sgi[o0 + s0 + n:o0 + s1] = (ss[0] - sub) if n else 0
                    rid[o0 + s0 + n:o0 + s1] = (rr[0] - q * P) if n else 0.0
            coff += s_los[b] + s_his[b]

        sgi_cols, doh_cols = [], []
        coff = 0
        for b in range(n_blocks):
            s_lo, s_hi, S = s_los[b], s_his[b], s_tot[b]
            o0 = coff * P
            lo_a = _idx_to_gather_layout(sgi[o0:o0 + s_lo * P])
            hi_a = (_idx_to_gather_layout(sgi[o0 + s_lo * P:o0 + S * P])
                    if s_hi > 0 else np.zeros((P, 0), np.int16))
            sgi_cols.append(np.concatenate([lo_a, hi_a], axis=1))
            doh_cols.append(doh[o0:o0 + S * P].reshape(S, P).T)
            coff += S
        sgi_l = np.concatenate(sgi_cols, axis=1)
        doh_l = np.concatenate(doh_cols, axis=1)
        dohT = np.broadcast_to(doh[None, :], (P, W_chunks * P))
        ridTs = []
        for p in range(n_par):
            rp = np.where(qof % n_par == p, rid.astype(np.float32),
                          -1.0).astype(np.float16)
            ridTs.append(np.ascontiguousarray(
                np.broadcast_to(rp[None, :], (P, W_chunks * P))))

        el = np.asarray(ent_emb, np.float32)[cidx * NPC:(cidx + 1) * NPC]
        ehi, elo = _bfsplit(el)

        in_maps.append({
            "ent": np.ascontiguousarray(ent_emb, np.float32),
            "ent_loc_hi": np.ascontiguousarray(ehi),
            "ent_loc_lo": np.ascontiguousarray(elo),
            "rel_hi": rlh,
            "rel_lo": rll,
            "w": np.ascontiguousarray(neigh_w, np.float32),
            "iota": iota,
            "iota_col16": iota_col16,
            "src_gi": np.ascontiguousarray(sgi_l),
            "dst_oh": np.ascontiguousarray(doh_l),
            "dst_ohT": np.ascontiguousarray(dohT),
            **{f"relidT{p}": ridTs[p] for p in range(n_par)},
        })
    key = (NPC, N_ENT, LO_ROWS, tuple(s_los), tuple(s_his), n_par,
           tuple(tuple(r) for r in sum(runs_all, [])))
    return in_maps, key, s_los, s_his, runs_all, n_par


LAST_RESULT = None


def _install_ntff_hook():
    import sys
    import types
    if "antenv.axon_hooks" in sys.modules:
        return
    mod = types.ModuleType("antenv.axon_hooks")
    hook = [None]
    mod.set_axon_ntff_profile_hook = lambda h: hook.__setitem__(0, h)
    mod.get_axon_ntff_profile_hook = lambda: hook[0]
    sys.modules["antenv.axon_hooks"] = mod
    import antenv
    antenv.axon_hooks = mod
    try:
        from trn_agent_boot.trn_boot import _ntff_profile_via_ctypes
        h = _ntff_profile_via_ctypes("/opt/axon/libaxon_pjrt.so")
        if h is not None:
            mod.set_axon_ntff_profile_hook(lambda *a, **k: h(*a, **k))
    except Exception as e:
        print("ntff hook install failed:", e)


def kernel(ent_emb, rel_emb, neigh_w, src, dst, rel_id, _trace=False):
    global LAST_RESULT
    from concourse.bass_utils import run_bass_kernel_spmd
    if _trace:
        _install_ntff_hook()

    in_maps, key, s_los, s_his, runs_all, n_par = _prep_inputs(
        ent_emb, rel_emb, neigh_w, src, dst, rel_id)
    if key not in _cache:
        _cache[key] = _build_program(NPC, N_ENT, LO_ROWS,
                                     s_los, s_his, runs_all, n_par)
    nc = _cache[key]
    res = run_bass_kernel_spmd(nc, in_maps, list(range(N_CORES)),
                               trace=_trace)
    LAST_RESULT = res
    return np.concatenate([r["out"] for r in res.results], axis=0)



# revision 3
# speedup vs baseline: 2.0184x; 2.0184x over previous
"""GNN message-passing (CompGCN edge-softmax) TRN2 kernel — no rel gather.

Same contract/sharding as kernel.py (edges sharded by dst range, 8 cores).

SWDGE gather costs ~8 ns/row on GPSIMD, so this version gathers ONLY
ent[src] (one row per edge).  The other two per-edge rows come from TensorE:

  dst rows:  dstrows[e,h] = sum_j OHT[j,e]*E_blk[j,h]   (E_blk = block's 128
             local dst rows, contiguous DMA, dual-bf16 hi+lo for exactness)
  rel rows:  relrows[e,h] = sum_r rOHT[r',e]*rel_q[r',h] (rel table SBUF-
             resident in 8 chunks of 128 rows, dual-bf16 hi+lo; edges are
             rel-sorted within each block section with per-(section,q) slot
             counts FIXED across cores, so each 128-slot chunk intersects a
             couple of compile-time-known q ranges -> partition-offset
             matmuls)

score = sum(comp * dstrows) stays fp32-exact (dual-bf16 residual ~1e-4).
Aggregation runs in bf16 (comp cast + W one-hot*es), err ~1e-2 < 2e-2 tol:
    accT[h,j] += comp_c.T @ W_c      (bf16 TensorE, fp32 PSUM)
    den[j]     = Wsum.T @ ones       (Wsum = sum_c W_c on DVE)
    out_block  = tanh((accT.T @ neigh_w)/den)
"""

import numpy as np

N_ENT = 50000
N_REL = 1000
H = 128
P = 128
NQ = 8                      # rel table chunks of 128 rows
N_CORES = 8
NPC = N_ENT // N_CORES
LO_ROWS = 32768

_cache = {}


def _bfsplit(x):
    import ml_dtypes
    hi = x.astype(np.float32).astype(ml_dtypes.bfloat16)
    lo = (x.astype(np.float32) - hi.astype(np.float32)).astype(ml_dtypes.bfloat16)
    return hi, lo


def _build_program(npc, n_ent, lo_rows, s_los, s_his, runs_all, n_par):
    """runs_all[b] = list of (c, q, s0, s1); q%n_par picks the parity
    one-hot tile whose rows are zero outside q's slots."""
    import concourse.bacc as bacc
    import concourse.mybir as mybir
    import concourse.tile as tile

    f32 = mybir.dt.float32
    f16 = mybir.dt.float16
    bf16 = mybir.dt.bfloat16
    i16 = mybir.dt.int16
    n_blocks = len(s_los)
    s_tot = [a + b for a, b in zip(s_los, s_his)]
    S_max = max(s_tot)
    W_chunks = sum(s_tot)
    W_slots = W_chunks * P

    nc = bacc.Bacc("TRN2", target_bir_lowering=False, debug=False,
                   num_devices=N_CORES, num_swdge_queues=4)

    ent = nc.dram_tensor("ent", [n_ent, H], f32, kind="ExternalInput")
    elh_in = nc.dram_tensor("ent_loc_hi", [npc, H], bf16, kind="ExternalInput")
    ell_in = nc.dram_tensor("ent_loc_lo", [npc, H], bf16, kind="ExternalInput")
    rlh_in = nc.dram_tensor("rel_hi", [P, NQ, H], bf16, kind="ExternalInput")
    rll_in = nc.dram_tensor("rel_lo", [P, NQ, H], bf16, kind="ExternalInput")
    w_in = nc.dram_tensor("w", [H, H], f32, kind="ExternalInput")
    iota_in = nc.dram_tensor("iota", [P, P], f16, kind="ExternalInput")
    ioc16_in = nc.dram_tensor("iota_col16", [P, 1], f16, kind="ExternalInput")
    sgi_in = nc.dram_tensor("src_gi", [P, W_chunks * 8], i16,
                            kind="ExternalInput")
    doh_in = nc.dram_tensor("dst_oh", [P, W_chunks], f16,
                            kind="ExternalInput")
    dohT_in = nc.dram_tensor("dst_ohT", [P, W_slots], f16,
                             kind="ExternalInput")
    ridT_in = [nc.dram_tensor(f"relidT{p}", [P, W_slots], f16,
                              kind="ExternalInput") for p in range(n_par)]
    out = nc.dram_tensor("out", [npc, H], f32, kind="ExternalOutput")

    import concourse.bass as bass

    def bc(ap, dims):
        return bass.AP(ap.tensor, ap.offset, dims)

    with tile.TileContext(nc) as tc:
        with (
            tc.tile_pool(name="const", bufs=1) as constp,
            tc.tile_pool(name="idx", bufs=1) as idxp,
            tc.tile_pool(name="data", bufs=2) as datap,
            tc.tile_pool(name="small", bufs=2) as smallp,
            tc.tile_pool(name="psum", bufs=1, space="PSUM") as psump,
            tc.tile_pool(name="psumr", bufs=2, space="PSUM") as psumrp,
            tc.tile_pool(name="psumb", bufs=1, space="PSUM") as psumbp,
        ):
            iota_t = constp.tile([P, P], f16)
            nc.sync.dma_start(iota_t[:], iota_in[:])
            ioc16_t = constp.tile([P, 1], f16)
            nc.sync.dma_start(ioc16_t[:], ioc16_in[:])
            w_t = constp.tile([H, H], f32)
            nc.sync.dma_start(w_t[:], w_in[:])
            rlh_t = constp.tile([P, NQ, H], bf16)
            nc.sync.dma_start(rlh_t[:], rlh_in[:])
            rll_t = constp.tile([P, NQ, H], bf16)
            nc.sync.dma_start(rll_t[:], rll_in[:])
            ones_bf = constp.tile([P, 1], bf16)
            nc.vector.memset(ones_bf[:], 1.0)

            sgi_t = idxp.tile([P, W_chunks * 8], i16)
            nc.sync.dma_start(sgi_t[:], sgi_in[:])
            doh_t = idxp.tile([P, W_chunks], f16)
            nc.sync.dma_start(doh_t[:], doh_in[:])

            coff = 0
            for b in range(n_blocks):
                base = b * P
                nodes_b = min(P, npc - base)
                s_lo, s_hi = s_los[b], s_his[b]
                S = s_lo + s_hi
                ns = S * P

                src_rows = datap.tile([P, S_max, H], f32, tag="src")
                relrows = datap.tile([P, S_max, H], f32, tag="relrows")
                w_oh = datap.tile([P, S_max, H], bf16, tag="W")
                comp_bf = datap.tile([P, S_max, H], bf16, tag="compbf")
                w_bf = datap.tile([P, S_max, H], bf16, tag="Wbf")
                oht_t = datap.tile([P, S_max * P], bf16, tag="OHT")
                roht_t = [datap.tile([P, S_max * P], bf16, tag=f"rOHT{p}",
                                     name=f"roht{p}") for p in range(n_par)]
                dohT_t = datap.tile([P, S_max * P], f16, tag="dohT")
                ridT_t = [datap.tile([P, S_max * P], f16, tag=f"ridT{p}",
                                     name=f"ridt{p}") for p in range(n_par)]
                ehi_t = datap.tile([P, H], bf16, tag="ehi")
                elo_t = datap.tile([P, H], bf16, tag="elo")

                if s_lo > 0:
                    nc.gpsimd.dma_gather(
                        src_rows[:, 0:s_lo, :], ent[0:lo_rows, :],
                        sgi_t[:, coff * 8:(coff + s_lo) * 8],
                        s_lo * P, s_lo * P, H, single_packet=False,
                        queue_num=(2 * b) % 4)
                if s_hi > 0:
                    nc.gpsimd.dma_gather(
                        src_rows[:, s_lo:S, :], ent[lo_rows:n_ent, :],
                        sgi_t[:, (coff + s_lo) * 8:(coff + S) * 8],
                        s_hi * P, s_hi * P, H, single_packet=False,
                        queue_num=(2 * b + 1) % 4)
                nc.sync.dma_start(dohT_t[:, 0:ns],
                                  dohT_in[:, coff * P:coff * P + ns])
                for p in range(n_par):
                    nc.sync.dma_start(ridT_t[p][:, 0:ns],
                                      ridT_in[p][:, coff * P:coff * P + ns])
                if nodes_b < P:
                    nc.vector.memset(ehi_t[:], 0.0)
                    nc.vector.memset(elo_t[:], 0.0)
                nc.sync.dma_start(ehi_t[:nodes_b, :],
                                  elh_in[base:base + nodes_b, :])
                nc.sync.dma_start(elo_t[:nodes_b, :],
                                  ell_in[base:base + nodes_b, :])

                # transposed one-hots (bf16 out)
                i16_ap = ioc16_t[:]
                nc.vector.tensor_tensor(
                    out=oht_t[:, 0:ns], in0=dohT_t[:, 0:ns],
                    in1=bc(i16_ap, [i16_ap.ap[0], [0, ns]]),
                    op=mybir.AluOpType.is_equal)
                for p in range(n_par):
                    nc.vector.tensor_tensor(
                        out=roht_t[p][:, 0:ns], in0=ridT_t[p][:, 0:ns],
                        in1=bc(i16_ap, [i16_ap.ap[0], [0, ns]]),
                        op=mybir.AluOpType.is_equal)

                # dstrows[e,h] = OHT_c.T @ (E_hi + E_lo)
                drows_ps = psumbp.tile([P, S_max, H], f32, tag="drows")
                for c in range(S):
                    lhs = oht_t[:, c * P:(c + 1) * P]
                    nc.tensor.matmul(drows_ps[:, c, :], lhsT=lhs,
                                     rhs=ehi_t[:], start=True, stop=False)
                    nc.tensor.matmul(drows_ps[:, c, :], lhsT=lhs,
                                     rhs=elo_t[:], start=False, stop=True)

                # relrows chunks: accumulate one (hi+lo) mm pair per q
                # present in the chunk, via its parity one-hot tile
                for c in range(S):
                    rel_ps = psumrp.tile([P, H], f32, tag="relps")
                    qs = [q for (cc, q, _s0, _s1) in runs_all[b] if cc == c]
                    for i, q in enumerate(qs):
                        lhs = roht_t[q % n_par][:, c * P:(c + 1) * P]
                        nc.tensor.matmul(rel_ps[:], lhsT=lhs,
                                         rhs=rlh_t[:, q, :],
                                         start=(i == 0), stop=False)
                        nc.tensor.matmul(rel_ps[:], lhsT=lhs,
                                         rhs=rll_t[:, q, :],
                                         start=False, stop=(i == len(qs) - 1))
                    nc.scalar.copy(relrows[:, c, :], rel_ps[:])

                # comp (fp32, in-place over src_rows) + bf16 cast for accT
                nc.vector.tensor_tensor(
                    out=src_rows[:, 0:S, :], in0=src_rows[:, 0:S, :],
                    in1=relrows[:, 0:S, :], op=mybir.AluOpType.mult)
                nc.scalar.copy(comp_bf[:, 0:S, :], src_rows[:, 0:S, :])

                # score = sum_h comp*dstrows  (prod scratch into relrows)
                nc.vector.tensor_tensor(
                    out=relrows[:, 0:S, :], in0=src_rows[:, 0:S, :],
                    in1=drows_ps[:, 0:S, :], op=mybir.AluOpType.mult)
                score = smallp.tile([P, S_max], f32, tag="score")
                nc.vector.tensor_reduce(
                    out=score[:, 0:S], in_=relrows[:, 0:S, :],
                    axis=mybir.AxisListType.X, op=mybir.AluOpType.add)
                es = smallp.tile([P, S_max], bf16, tag="es")
                nc.scalar.activation(
                    out=es[:, 0:S], in_=score[:, 0:S],
                    func=mybir.ActivationFunctionType.Exp)

                # W one-hot (fp32) * es -> bf16
                doh_ap = doh_t[:, coff:coff + S]
                doh_b = bc(doh_ap, [doh_ap.ap[0], doh_ap.ap[1], [0, H]])
                iota_ap = iota_t[:]
                iota_b = bc(iota_ap, [iota_ap.ap[0], [0, S], iota_ap.ap[1]])
                nc.vector.tensor_tensor(
                    out=w_oh[:, 0:S, :], in0=doh_b, in1=iota_b,
                    op=mybir.AluOpType.is_equal)
                es_ap = es[:, 0:S]
                es_b = bc(es_ap, [es_ap.ap[0], es_ap.ap[1], [0, H]])
                nc.vector.tensor_tensor(
                    out=w_bf[:, 0:S, :], in0=w_oh[:, 0:S, :], in1=es_b,
                    op=mybir.AluOpType.mult)

                # den[j] = sum_c W_c.T @ ones  (bf16 TensorE)
                ps_m = psump.tile([P, H], f32, tag="misc")
                for c in range(S):
                    nc.tensor.matmul(ps_m[:, 0:1], lhsT=w_bf[:, c, :],
                                     rhs=ones_bf[:],
                                     start=(c == 0), stop=(c == S - 1))

                # accT[h,j] += comp_c.T @ W_c  (bf16)
                acct_ps = psump.tile([P, P], f32, tag="accT")
                for c in range(S):
                    nc.tensor.matmul(
                        acct_ps[:], lhsT=comp_bf[:, c, :], rhs=w_bf[:, c, :],
                        start=(c == 0), stop=(c == S - 1))

                acct_sb = smallp.tile([P, P], f32, tag="acct_sb")
                nc.scalar.copy(acct_sb[:], acct_ps[:])
                den_sb = smallp.tile([P, 1], f32, tag="den_sb")
                nc.vector.tensor_scalar_max(den_sb[:], ps_m[:, 0:1], 1e-30)
                rden = smallp.tile([P, 1], f32, tag="rden")
                nc.vector.reciprocal(rden[:], den_sb[:])

                nc.tensor.matmul(ps_m[:], lhsT=acct_sb[:], rhs=w_t[:],
                                 start=True, stop=True)
                out_sb = smallp.tile([P, H], f32, tag="out_sb")
                nc.scalar.activation(
                    out=out_sb[:], in_=ps_m[:],
                    func=mybir.ActivationFunctionType.Tanh, scale=rden[:])
                nc.sync.dma_start(out[base:base + nodes_b, :],
                                  out_sb[:nodes_b, :])
                coff += S

    nc.compile()
    return nc


def _idx_to_gather_layout(arr):
    a = arr.reshape(-1, 16).T.astype(np.int16)
    return np.tile(a, (8, 1))


def _prep_inputs(ent_emb, rel_emb, neigh_w, src, dst, rel_id):
    """Edges by dst block; rel-q-partitioned sections with core-uniform
    slot layout; build src gather idx + one-hot index maps."""
    import ml_dtypes
    src = np.asarray(src).astype(np.int64)
    dst = np.asarray(dst).astype(np.int64)
    rel_id = np.asarray(rel_id).astype(np.int64)
    n_blocks = (NPC + P - 1) // P

    order = np.argsort(dst, kind="stable")
    src_s, dst_s, rel_s = src[order], dst[order], rel_id[order]
    g_s = (dst_s // NPC) * n_blocks + (dst_s % NPC) // P
    n_gblocks = N_CORES * n_blocks
    bounds = np.searchsorted(g_s, np.arange(n_gblocks + 1))

    # per (core, block, section, q) edge lists
    # section 0 = src<LO_ROWS, 1 = src>=LO_ROWS
    per = {}
    for c in range(N_CORES):
        for b in range(n_blocks):
            e0, e1 = bounds[c * n_blocks + b], bounds[c * n_blocks + b + 1]
            s_g, d_g, r_g = src_s[e0:e1], dst_s[e0:e1], rel_s[e0:e1]
            sec = (s_g >= LO_ROWS).astype(np.int64)
            q_g = r_g // P
            for s in (0, 1):
                for q in range(NQ):
                    m = (sec == s) & (q_g == q)
                    per[(c, b, s, q)] = (s_g[m], d_g[m], r_g[m])

    # core-uniform slot counts per (block, section, q), 16-aligned for tidiness
    cnt = {}
    for b in range(n_blocks):
        for s in (0, 1):
            for q in range(NQ):
                m = max(len(per[(c, b, s, q)][0]) for c in range(N_CORES))
                if s == 0 and q == 0:
                    m = max(m, 1)
                cnt[(b, s, q)] = m

    # slot layout per block: lo qs, pad to 128; hi qs, pad to 128
    s_los, s_his, runs_all, layouts = [], [], [], []
    for b in range(n_blocks):
        lo_n = sum(cnt[(b, 0, q)] for q in range(NQ))
        hi_n = sum(cnt[(b, 1, q)] for q in range(NQ))
        s_lo = max((lo_n + P - 1) // P, 1)
        s_hi = (hi_n + P - 1) // P
        s_los.append(s_lo)
        s_his.append(s_hi)
        # slot ranges: [(q, s0, s1, sec)] in slot order; section pads get
        # q = last q of the section (repeat edges carry that q's relid)
        lay = []
        pos = 0
        for s, sbase, stot in ((0, 0, s_lo), (1, s_lo, s_hi)):
            pos = sbase * P
            for q in range(NQ):
                n = cnt[(b, s, q)]
                if n:
                    lay.append((q, pos, pos + n, s, False))
                    pos += n
            end = (sbase + stot) * P
            if pos < end and lay:
                lq, ls0, _ls1, lsec, lpad = lay[-1]
                if lsec == s and not lpad:
                    # extend the last real range; fill pads it with repeats
                    lay[-1] = (lq, ls0, end, s, False)
                else:
                    lay.append((lq, pos, end, s, True))
        layouts.append(lay)
        # runs: intersect layout ranges with 128-chunks
        runs = []
        for (q, s0, s1, _sec, _pad) in lay:
            c0, c1 = s0 // P, (s1 - 1) // P
            for c in range(c0, c1 + 1):
                a = max(s0, c * P)
                z = min(s1, (c + 1) * P)
                if a < z:
                    runs.append((c, q, a, z))
        runs_all.append(runs)
    s_tot = [a + b for a, b in zip(s_los, s_his)]
    W_chunks = sum(s_tot)

    iota = np.broadcast_to(np.arange(P, dtype=np.float16), (P, P)).copy()
    iota_col16 = np.arange(P, dtype=np.float16).reshape(P, 1).copy()

    # smallest parity count with no per-chunk q%k collision
    n_par = 3
    while True:
        ok = True
        for runs in runs_all:
            from collections import defaultdict
            byc = defaultdict(list)
            for (c, q, _s0, _s1) in runs:
                byc[c].append(q % n_par)
            if any(len(v) != len(set(v)) for v in byc.values()):
                ok = False
                break
        if ok:
            break
        n_par += 1
    rel_pad = np.zeros((NQ * P, H), np.float32)
    rel_pad[:N_REL] = np.asarray(rel_emb, np.float32)
    rhi, rlo = _bfsplit(rel_pad)
    rlh = np.ascontiguousarray(rhi.reshape(NQ, P, H).transpose(1, 0, 2))
    rll = np.ascontiguousarray(rlo.reshape(NQ, P, H).transpose(1, 0, 2))

    in_maps = []
    for cidx in range(N_CORES):
        sgi = np.zeros((W_chunks * P,), np.int16)
        doh = np.full((W_chunks * P,), float(P), np.float16)
        rid = np.zeros((W_chunks * P,), np.float16)
        qof = np.full((W_chunks * P,), -1, np.int64)
        coff = 0
        for b in range(n_blocks):
            o0 = coff * P
            for (q, s0, s1, sec, is_pad) in layouts[b]:
                ss, dd, rr = per[(cidx, b, sec, q)]
                if is_pad:
                    ss = ss[:0]
                    dd = dd[:0]
                    rr = rr[:0]
                n = len(ss)
                cap = s1 - s0
                assert n <= cap
                sub = LO_ROWS if sec == 1 else 0
                base = cidx * NPC + b * P
                qof[o0 + s0:o0 + s1] = q
                if n:
                    sgi[o0 + s0:o0 + s0 + n] = ss - sub
                    doh[o0 + s0:o0 + s0 + n] = (dd - base).astype(np.float16)
                    rid[o0 + s0:o0 + s0 + n] = (rr - q * P).astype(np.float16)
                if n < cap:
                    # pad: repeat a real edge of this q (doh stays 128),
                    # or r'=0 of this q if the core has none
                    sgi[o0 + s0 + n:o0 + s1] = (ss[0] - sub) if n else 0
                    rid[o0 + s0 + n:o0 + s1] = (rr[0] - q * P) if n else 0.0
            coff += s_los[b] + s_his[b]

        sgi_cols, doh_cols = [], []
        coff = 0
        for b in range(n_blocks):
            s_lo, s_hi, S = s_los[b], s_his[b], s_tot[b]
            o0 = coff * P
            lo_a = _idx_to_gather_layout(sgi[o0:o0 + s_lo * P])
            hi_a = (_idx_to_gather_layout(sgi[o0 + s_lo * P:o0 + S * P])
                    if s_hi > 0 else np.zeros((P, 0), np.int16))
            sgi_cols.append(np.concatenate([lo_a, hi_a], axis=1))
            doh_cols.append(doh[o0:o0 + S * P].reshape(S, P).T)
            coff += S
        sgi_l = np.concatenate(sgi_cols, axis=1)
        doh_l = np.concatenate(doh_cols, axis=1)
        dohT = np.broadcast_to(doh[None, :], (P, W_chunks * P))
        ridTs = []
        for p in range(n_par):
            rp = np.where(qof % n_par == p, rid.astype(np.float32),
                          -1.0).astype(np.float16)
            ridTs.append(np.ascontiguousarray(
                np.broadcast_to(rp[None, :], (P, W_chunks * P))))

        el = np.asarray(ent_emb, np.float32)[cidx * NPC:(cidx + 1) * NPC]
        ehi, elo = _bfsplit(el)

        in_maps.append({
            "ent": np.ascontiguousarray(ent_emb, np.float32),
            "ent_loc_hi": np.ascontiguousarray(ehi),
            "ent_loc_lo": np.ascontiguousarray(elo),
            "rel_hi": rlh,
            "rel_lo": rll,
            "w": np.ascontiguousarray(neigh_w, np.float32),
            "iota": iota,
            "iota_col16": iota_col16,
            "src_gi": np.ascontiguousarray(sgi_l),
            "dst_oh": np.ascontiguousarray(doh_l),
            "dst_ohT": np.ascontiguousarray(dohT),
            **{f"relidT{p}": ridTs[p] for p in range(n_par)},
        })
    key = (NPC, N_ENT, LO_ROWS, tuple(s_los), tuple(s_his), n_par,
           tuple(tuple(r) for r in sum(runs_all, [])))
    return in_maps, key, s_los, s_his, runs_all, n_par


LAST_RESULT = None


def _install_ntff_hook():
    import sys
    import types
    if "antenv.axon_hooks" in sys.modules:
        return
    mod = types.ModuleType("antenv.axon_hooks")
    hook = [None]
    mod.set_axon_ntff_profile_hook = lambda h: hook.__setitem__(0, h)
    mod.get_axon_ntff_profile_hook = lambda: hook[0]
    sys.modules["antenv.axon_hooks"] = mod
    import antenv
    antenv.axon_hooks = mod
    try:
        from trn_agent_boot.trn_boot import _ntff_profile_via_ctypes
        h = _ntff_profile_via_ctypes("/opt/axon/libaxon_pjrt.so")
        if h is not None:
            mod.set_axon_ntff_profile_hook(lambda *a, **k: h(*a, **k))
    except Exception as e:
        print("ntff hook install failed:", e)


def kernel(ent_emb, rel_emb, neigh_w, src, dst, rel_id, _trace=False):
    global LAST_RESULT
    from concourse.bass_utils import run_bass_kernel_spmd
    if _trace:
        _install_ntff_hook()

    in_maps, key, s_los, s_his, runs_all, n_par = _prep_inputs(
        ent_emb, rel_emb, neigh_w, src, dst, rel_id)
    if key not in _cache:
        _cache[key] = _build_program(NPC, N_ENT, LO_ROWS,
                                     s_los, s_his, runs_all, n_par)
    nc = _cache[key]
    res = run_bass_kernel_spmd(nc, in_maps, list(range(N_CORES)),
                               trace=_trace)
    LAST_RESULT = res
    return np.concatenate([r["out"] for r in res.results], axis=0)

